# revision 24
# baseline (speedup 1.0000x reference)
"""Causal self-attention (B=2, S=2048, D=1024, H=16) on 8 Trainium2 cores.

Sharding: batch x head-group. Core c handles batch c//4 and heads
[4*(c%4), 4*(c%4)+4). Each core computes q/k/v projections for its head
slice, causal flash-attention (transposed layout, no max-subtraction --
scores are bounded ~9), and a row-parallel partial output projection in
bf16. The host transposes/sums the 8 partial outputs and adds b_proj.

Schedule: s-chunks processed causally; the softmax exp stream (Scalar
engine) paces attention, so all projection matmuls for the NEXT chunk
are chopped into ~0.9us filler units and interleaved into the attention
pairs to keep the PE gapless.
"""

import sys

import numpy as np

try:
    import concourse.bass as bass  # noqa: F401
except ImportError:  # fallback for environments without the site hook
    sys.path.insert(0, "/opt/trn_rl_repo")

import concourse.bacc as bacc
import concourse.bass as bass
import concourse.mybir as mybir
from concourse import tile
from concourse.bass_utils import run_bass_kernel_spmd

B, S, D, H = 2, 2048, 1024, 16
HD = D // H  # 64
SCALE = 1.0 / np.sqrt(HD)  # 0.125
HPC = 4          # heads per core
NCORES = 8
P = 128          # partitions
QC = 512         # query chunk (matmul free dim)
NQ = S // QC     # 4 query chunks
NK = S // P      # 16 key tiles
ND = D // P      # 8 d tiles
F32 = mybir.dt.float32
BF16 = mybir.dt.bfloat16
VW = HPC * (HD + 1)  # 260 cols of augmented v (per-head 64 v dims + ones)
VP = 336             # v stride so every head slice can read a 128-col lhsT

_PROGRAM = None


def _build_program():
    """Build the SPMD Bass program (same NEFF for all 8 cores)."""
    nc = bacc.Bacc(None, target_bir_lowering=False)

    xt = nc.declare_dram_parameter("xt", [D, S], BF16, isOutput=False)
    wqk = nc.declare_dram_parameter("wqk", [D, 4 * P], BF16, isOutput=False)
    bqk = nc.declare_dram_parameter("bqk", [P, 4], F32, isOutput=False)
    wv = nc.declare_dram_parameter("wv", [D, VW], BF16, isOutput=False)
    wvb = nc.declare_dram_parameter("wvb", [1, VW], BF16, isOutput=False)
    tri = nc.declare_dram_parameter("tri", [P, 2 * P], BF16, isOutput=False)
    wp = nc.declare_dram_parameter("wp", [HPC * HD, D], BF16, isOutput=False)
    yt = nc.declare_dram_parameter("yt", [D, S], BF16, isOutput=True)

    with tile.TileContext(nc) as tc:
        with (
            tc.tile_pool(name="const", bufs=1) as const,
            tc.tile_pool(name="big", bufs=1) as bigp,
            tc.tile_pool(name="work", bufs=4) as work,
            tc.tile_pool(name="small", bufs=2) as small,
            tc.tile_pool(name="ps", bufs=2, space="PSUM") as psp,
        ):
            # ---- PE warm-up: a few matmuls on memset data so the HAM
            # clock-gate opens while the first DMAs land ----
            wup = const.tile([P, QC], BF16, tag="wup")
            nc.vector.memset(wup[:], 0)
            for i in range(4):
                wps = psp.tile([P, QC], F32, tag="pv", name=f"wps{i}")
                nc.tensor.matmul(wps[:], wup[:, 0:P], wup[:], start=True, stop=True)

            # ---- DMA prologue: wqk/xt(sc0) interleaved so the first
            # qk-projection can start as early as possible ----
            wqk_sb = const.tile([P, ND * 4 * P], BF16, tag="wqk")
            xt_sb = bigp.tile([P, ND * S], BF16, tag="xt")

            def xts(dt):
                return xt_sb[:, dt * S:(dt + 1) * S]

            def wqks(dt):
                return wqk_sb[:, dt * 4 * P:(dt + 1) * 4 * P]

            # tiny high-priority transfers first: the first drains/masks
            # need them and they barely delay the big stream
            bqk_sb = const.tile([P, 4], F32, tag="bqk")
            nc.sync.dma_start(bqk_sb[:], bqk[:])
            wvb_sb = const.tile([1, VW], BF16, tag="wvb")
            nc.sync.dma_start(wvb_sb[:], wvb[:])
            tri_sb = const.tile([P, 2 * P], BF16, tag="tri")
            nc.sync.dma_start(tri_sb[:], tri[:])
            for dt in range(ND):
                nc.sync.dma_start(wqks(dt), wqk[dt * P:(dt + 1) * P, :])
                nc.sync.dma_start(
                    xts(dt)[:, 0:QC], xt[dt * P:(dt + 1) * P, 0:QC]
                )
            wv_sb = const.tile([P, ND * VW], BF16, tag="wv")
            for dt in range(ND):
                nc.sync.dma_start(
                    wv_sb[:, dt * VW:(dt + 1) * VW], wv[dt * P:(dt + 1) * P, :]
                )
            # xt chunk 1 early (vproj 4..7 is filler inside attention qt=0)
            for dt in range(ND):
                nc.sync.dma_start(
                    xts(dt)[:, QC:2 * QC], xt[dt * P:(dt + 1) * P, QC:2 * QC]
                )
            wp_sb = const.tile([P, 2 * D], BF16, tag="wp")
            for i in range(2):
                nc.sync.dma_start(
                    wp_sb[:, i * D:(i + 1) * D], wp[i * P:(i + 1) * P, :]
                )
            for sc in range(2, NQ):
                for dt in range(ND):
                    nc.sync.dma_start(
                        xts(dt)[:, sc * QC:(sc + 1) * QC],
                        xt[dt * P:(dt + 1) * P, sc * QC:(sc + 1) * QC],
                    )

            # v bias (+ ones columns) broadcast across partitions, once
            vbb_sb = const.tile([P, VW], BF16, tag="vbb")
            nc.gpsimd.partition_broadcast(vbb_sb[:], wvb_sb[:])

            # ---- persistent intermediates ----
            qt_sb = [bigp.tile([P, S], BF16, tag=f"qt{i}", name=f"qt{i}") for i in range(2)]
            kt_sb = [bigp.tile([P, S], BF16, tag=f"kt{i}", name=f"kt{i}") for i in range(2)]
            v_sb = bigp.tile([P, NK * VP], BF16, tag="v")
            ot_sb = [bigp.tile([P, S], BF16, tag=f"ot{i}", name=f"ot{i}") for i in range(2)]

            def qk_half(sc, et, half):
                """Half of one qk-projection tile (4 of 8 d-tiles)."""
                if half == 0:
                    qk_half.ps = psp.tile(
                        [P, QC], F32, tag="proj", name=f"qk{sc}{et}"
                    )
                ps = qk_half.ps
                for dt in range(4 * half, 4 * half + 4):
                    nc.tensor.matmul(
                        ps[:],
                        wqks(dt)[:, et * P:(et + 1) * P],
                        xts(dt)[:, sc * QC:(sc + 1) * QC],
                        start=(dt == 0),
                        stop=(dt == ND - 1),
                    )
                if half == 1:
                    dest = (qt_sb if et < 2 else kt_sb)[et % 2]
                    if sc <= 1:
                        # front of the kernel: exp stream is idle, DVE busy
                        nc.scalar.activation(
                            dest[:, sc * QC:(sc + 1) * QC], ps[:],
                            mybir.ActivationFunctionType.Identity,
                            bias=bqk_sb[:, et:et + 1],
                        )
                    else:
                        nc.vector.tensor_scalar_add(
                            dest[:, sc * QC:(sc + 1) * QC], ps[:],
                            bqk_sb[:, et:et + 1],
                        )

            def emit_v(st):
                ps = psp.tile([P, VW], F32, tag="proj", name=f"vp{st}")
                for dt in range(ND):
                    nc.tensor.matmul(
                        ps[:],
                        xts(dt)[:, st * P:(st + 1) * P],
                        wv_sb[:, dt * VW:(dt + 1) * VW],
                        start=(dt == 0),
                        stop=(dt == ND - 1),
                    )
                nc.vector.tensor_add(
                    v_sb[:, st * VP:st * VP + VW], ps[:], vbb_sb[:]
                )

            def mk_proj(qt, et, tag="proj", eng="v"):
                def f():
                    ps = psp.tile([P, QC], F32, tag=tag, name=f"yp{qt}{et}")
                    for i in range(2):
                        nc.tensor.matmul(
                            ps[:],
                            wp_sb[:, i * D + et * P:i * D + (et + 1) * P],
                            ot_sb[i][:, qt * QC:(qt + 1) * QC],
                            start=(i == 0),
                            stop=(i == 1),
                        )
                    ys = small.tile([P, QC], BF16, tag="ys", name=f"ys{qt}{et}",
                                    bufs=3)
                    if eng == "v":
                        nc.vector.tensor_copy(ys[:], ps[:])
                    else:
                        nc.scalar.copy(ys[:], ps[:])
                    nc.sync.dma_start(yt[et * P:(et + 1) * P, qt * QC:(qt + 1) * QC], ys[:])
                return f

            def emit_pair(qt, pair, fillers=()):
                """Attention for q-chunk qt, head-pair `pair` (2 heads)."""
                fillers = list(fillers)
                q0 = qt * QC
                nk = (qt + 1) * (QC // P)  # causal: k tiles 0..nk-1
                ht = pair
                pvs = [
                    psp.tile([P, QC], F32, tag="pv", name=f"pv{qt}{pair}{hh}")
                    for hh in range(2)
                ]
                exs = {}

                def emit_scores(kb):
                    j = kb - qt * (QC // P)
                    off = 0 if j < 0 else P * j
                    st2 = psp.tile(
                        [P, 2 * QC], F32, tag="sc", name=f"st{qt}{pair}{kb}"
                    )
                    for hh in range(2):
                        nc.tensor.matmul(
                            st2[:, hh * QC + off:(hh + 1) * QC],
                            kt_sb[ht][slice(64 * hh, 64 * hh + 64),
                                      kb * P:(kb + 1) * P],
                            qt_sb[ht][slice(64 * hh, 64 * hh + 64),
                                      q0 + off:q0 + QC],
                            start=True, stop=True,
                            tile_position=(64 * hh, 0),
                        )
                    ex = work.tile(
                        [P, 2 * QC], BF16, tag="ex", name=f"ex{qt}{pair}{kb}"
                    )
                    st3 = st2[:].rearrange("p (h q) -> p h q", h=2)[:, :, off:]
                    ex3 = ex[:].rearrange("p (h q) -> p h q", h=2)[:, :, off:]
                    nc.scalar.activation(
                        ex3, st3,
                        mybir.ActivationFunctionType.Exp,
                        scale=float(SCALE),
                    )
                    if j >= 0:
                        # only the 128-wide diagonal boundary needs masking
                        exb = ex[:].rearrange("p (h q) -> p h q", h=2)[:, :, off:off + P]
                        tri3 = tri_sb[:].rearrange("p (h q) -> p h q", h=2)
                        nc.vector.tensor_mul(exb, exb, tri3)
                    exs[kb] = ex

                def emit_pv(kb):
                    j = kb - qt * (QC // P)
                    off = 0 if j < 0 else P * j
                    ex = exs.pop(kb)
                    for hh in range(2):
                        h = 2 * pair + hh
                        nc.tensor.matmul(
                            pvs[hh][:, off:],
                            v_sb[:, kb * VP + h * (HD + 1):kb * VP + h * (HD + 1) + P],
                            ex[:, hh * QC + off:(hh + 1) * QC],
                            start=(kb == 0),
                            stop=(kb == nk - 1),
                        )

                # software pipeline: scores run 2 ahead of pv; one filler
                # after every other pv keeps the PE fed while exp paces
                emit_scores(0)
                if fillers:
                    fillers.pop(0)()
                emit_scores(1)
                for kb in range(nk):
                    if kb + 2 < nk:
                        emit_scores(kb + 2)
                    emit_pv(kb)
                    if fillers and kb % 2 == 0:
                        fillers.pop(0)()
                for f in fillers:
                    f()

                last = (qt, pair) == (NQ - 1, 1)
                dcps = []
                for hh in range(2):
                    # rows 0..63 are o^T, row 64 is the denominator
                    # (reciprocal_approx_fast misreads PSUM -> copy first)
                    dcp = small.tile(
                        [1, QC], F32, tag="dcp", name=f"dcp{qt}{pair}{hh}"
                    )
                    if last and hh == 0:
                        # final chain is fully exposed: run the two dcp
                        # copies on different engines in parallel
                        nc.scalar.copy(dcp[:], pvs[hh][HD:HD + 1, :])
                    else:
                        nc.vector.tensor_copy(dcp[:], pvs[hh][HD:HD + 1, :])
                    dcps.append(dcp)
                for hh in range(2):
                    rden = small.tile(
                        [1, QC], F32, tag="rden", name=f"rden{qt}{pair}{hh}"
                    )
                    nc.vector.reciprocal_approx_fast(rden[:], dcps[hh][:])
                    bden = small.tile(
                        [64, QC], F32, tag="bden", name=f"bden{qt}{pair}{hh}"
                    )
                    nc.gpsimd.partition_broadcast(bden[:], rden[:])
                    nc.vector.tensor_mul(
                        ot_sb[ht][slice(64 * hh, 64 * hh + 64), q0:q0 + QC],
                        pvs[hh][0:HD, :], bden[:],
                    )

            def qkproj(sc):
                for et in range(4):
                    qk_half(sc, et, 0)
                    qk_half(sc, et, 1)

            def mk_qk(sc, et, half):
                return lambda: qk_half(sc, et, half)

            # ---- causal streaming schedule ----
            # pair(x, 0) consumes et0/et2 (its own heads); its fillers
            # produce what the NEXT pair needs: pair(x,1) needs et1/et3,
            # pair(x+1,0) needs qkproj(x+1) et0/et2, plus v stripes and
            # outproj tiles of completed chunks.
            def mk_v(st):
                return lambda: emit_v(st)

            qkproj(0)
            for st in range(4):
                emit_v(st)
            emit_pair(0, 0, [mk_qk(1, et, h) for et in (0, 1) for h in (0, 1)]
                      + [mk_v(4)])
            emit_pair(0, 1, [mk_qk(1, et, h) for et in (2, 3) for h in (0, 1)]
                      + [mk_v(5)])
            emit_pair(1, 0,
                      [mk_v(6), mk_v(7)]
                      + [mk_qk(2, et, h) for et in (0, 1) for h in (0, 1)]
                      + [mk_proj(0, et) for et in range(2)])
            emit_pair(1, 1,
                      [mk_v(8), mk_v(9)]
                      + [mk_qk(2, et, h) for et in (2, 3) for h in (0, 1)]
                      + [mk_proj(0, et) for et in range(2, 4)])
            emit_pair(2, 0,
                      [mk_v(10), mk_v(11)]
                      + [mk_qk(3, et, h) for et in (0, 1) for h in (0, 1)]
                      + [mk_proj(0, et) for et in range(4, 6)]
                      + [mk_proj(1, et) for et in range(2)])
            emit_pair(2, 1,
                      [mk_v(12), mk_v(13)]
                      + [mk_qk(3, et, h) for et in (2, 3) for h in (0, 1)]
                      + [mk_proj(0, et) for et in range(6, 8)]
                      + [mk_proj(1, et) for et in range(2, 4)])
            emit_pair(3, 0,
                      [mk_v(14), mk_v(15)]
                      + [mk_proj(1, et) for et in range(4, 8)]
                      + [mk_proj(2, et) for et in range(2)])
            emit_pair(3, 1, [mk_proj(2, et) for et in range(2, 8)])
            # tail: the i=0 (first head-pair) halves of the first four
            # out-projection tiles run while the final normalize chain is
            # still draining; then finish with alternating psum tags and
            # drain engines so the last tiles stream back-to-back
            tail_ps = []
            for et in range(4):
                ps = psp.tile([P, QC], F32, tag=("proj", "sc")[et % 2],
                              name=f"yp3{et}")
                nc.tensor.matmul(
                    ps[:], wp_sb[:, et * P:(et + 1) * P],
                    ot_sb[0][:, 3 * QC:4 * QC], start=True, stop=False,
                )
                tail_ps.append(ps)
            for et in range(4):
                ps = tail_ps[et]
                nc.tensor.matmul(
                    ps[:], wp_sb[:, D + et * P:D + (et + 1) * P],
                    ot_sb[1][:, 3 * QC:4 * QC], start=False, stop=True,
                )
                ys = small.tile([P, QC], BF16, tag="ys", name=f"ys3{et}",
                                bufs=3)
                if et % 2 == 0:
                    nc.vector.tensor_copy(ys[:], ps[:])
                else:
                    nc.scalar.copy(ys[:], ps[:])
                nc.sync.dma_start(yt[et * P:(et + 1) * P, 3 * QC:4 * QC], ys[:])
            for et in range(4, 8):
                mk_proj(3, et, tag=("proj", "sc")[et % 2],
                        eng=("v", "s")[et % 2])()

    nc.compile()
    return nc


def _shard_inputs(x, w_qkv, b_qkv, w_proj):
    """Build the per-core input maps."""
    import ml_dtypes
    mdt = ml_dtypes.bfloat16

    in_maps = []
    kk = np.arange(P)[:, None]
    qq = np.arange(P)[None, :]
    tri_np = np.concatenate([(qq >= kk).astype(mdt)] * 2, axis=1)  # [128, 256]
    for c in range(NCORES):
        b, g = divmod(c, 4)
        e0 = g * HPC * HD  # 256*g
        xt_np = np.ascontiguousarray(x[b].T)
        q_rows = w_qkv[e0:e0 + HPC * HD]            # [256, 1024]
        k_rows = w_qkv[D + e0:D + e0 + HPC * HD]
        wqk_np = np.concatenate([q_rows.T, k_rows.T], 1)  # [1024, 512]
        wv_np = np.zeros((D, VW), np.float32)
        wvb_np = np.zeros((1, VW), np.float32)
        for h in range(HPC):
            rows = 2 * D + e0 + h * HD
            wv_np[:, h * (HD + 1):h * (HD + 1) + HD] = w_qkv[rows:rows + HD].T
            wvb_np[0, h * (HD + 1):h * (HD + 1) + HD] = b_qkv[rows:rows + HD]
            wvb_np[0, h * (HD + 1) + HD] = 1.0
        bqk_np = np.stack(
            [b_qkv[e0:e0 + P], b_qkv[e0 + P:e0 + 2 * P],
             b_qkv[D + e0:D + e0 + P], b_qkv[D + e0 + P:D + e0 + 2 * P]], 1
        ).astype(np.float32)
        wp_np = np.ascontiguousarray(w_proj[:, e0:e0 + HPC * HD].T)  # [256, 1024]
        in_maps.append({
            "xt": np.ascontiguousarray(xt_np.astype(mdt)),
            "wqk": np.ascontiguousarray(wqk_np.astype(mdt)),
            "bqk": np.ascontiguousarray(bqk_np),
            "wv": wv_np.astype(mdt),
            "wvb": wvb_np.astype(mdt),
            "tri": tri_np,
            "wp": wp_np.astype(mdt),
        })
    return in_maps


def _run(inputs, trace=False, trace_kwargs=None):
    global _PROGRAM
    if _PROGRAM is None:
        _PROGRAM = _build_program()
    nc = _PROGRAM
    x = np.asarray(inputs["x"], np.float32)
    w_qkv = np.asarray(inputs["w_qkv"], np.float32)
    b_qkv = np.asarray(inputs["b_qkv"], np.float32)
    w_proj = np.asarray(inputs["w_proj"], np.float32)
    b_proj = np.asarray(inputs["b_proj"], np.float32)
    in_maps = _shard_inputs(x, w_qkv, b_qkv, w_proj)
    res = run_bass_kernel_spmd(
        nc, in_maps, core_ids=list(range(NCORES)),
        trace=trace, **(trace_kwargs or {}),
    )
    y = np.zeros((B, S, D), np.float32)
    for c in range(NCORES):
        y[c // 4] += res.results[c]["yt"].T.astype(np.float32)
    y += b_proj
    return y, res


def kernel(**inputs):
    y, _ = _run(inputs)
    return y


# revision 25
# speedup vs baseline: 1.0083x; 1.0083x over previous
"""Causal self-attention (B=2, S=2048, D=1024, H=16) on 8 Trainium2 cores.

Sharding: batch x head-group. Core c handles batch c//4 and heads
[4*(c%4), 4*(c%4)+4). Each core computes q/k/v projections for its head
slice, causal flash-attention (transposed layout, no max-subtraction --
scores are bounded ~9), and a row-parallel partial output projection in
bf16. The host transposes/sums the 8 partial outputs and adds b_proj.

Schedule: s-chunks processed causally; the softmax exp stream (Scalar
engine) paces attention, so all projection matmuls for the NEXT chunk
are chopped into ~0.9us filler units and interleaved into the attention
pairs to keep the PE gapless.
"""

import sys

import numpy as np

try:
    import concourse.bass as bass  # noqa: F401
except ImportError:  # fallback for environments without the site hook
    sys.path.insert(0, "/opt/trn_rl_repo")

import concourse.bacc as bacc
import concourse.bass as bass
import concourse.mybir as mybir
from concourse import tile
from concourse.bass_utils import run_bass_kernel_spmd

B, S, D, H = 2, 2048, 1024, 16
HD = D // H  # 64
SCALE = 1.0 / np.sqrt(HD)  # 0.125
HPC = 4          # heads per core
NCORES = 8
P = 128          # partitions
QC = 512         # query chunk (matmul free dim)
NQ = S // QC     # 4 query chunks
NK = S // P      # 16 key tiles
ND = D // P      # 8 d tiles
F32 = mybir.dt.float32
BF16 = mybir.dt.bfloat16
VW = HPC * (HD + 1)  # 260 cols of augmented v (per-head 64 v dims + ones)
VP = 336             # v stride so every head slice can read a 128-col lhsT

_PROGRAM = None


def _build_program():
    """Build the SPMD Bass program (same NEFF for all 8 cores)."""
    nc = bacc.Bacc(None, target_bir_lowering=False)

    xt = nc.declare_dram_parameter("xt", [D, S], BF16, isOutput=False)
    wqk = nc.declare_dram_parameter("wqk", [D, 4 * P], BF16, isOutput=False)
    bqk = nc.declare_dram_parameter("bqk", [P, 4], F32, isOutput=False)
    wv = nc.declare_dram_parameter("wv", [D, VW], BF16, isOutput=False)
    wvb = nc.declare_dram_parameter("wvb", [1, VW], BF16, isOutput=False)
    tri = nc.declare_dram_parameter("tri", [P, 2 * P], BF16, isOutput=False)
    wp = nc.declare_dram_parameter("wp", [HPC * HD, D], BF16, isOutput=False)
    yt = nc.declare_dram_parameter("yt", [D, S], BF16, isOutput=True)

    with tile.TileContext(nc) as tc:
        with (
            tc.tile_pool(name="const", bufs=1) as const,
            tc.tile_pool(name="big", bufs=1) as bigp,
            tc.tile_pool(name="work", bufs=4) as work,
            tc.tile_pool(name="small", bufs=2) as small,
            tc.tile_pool(name="ps", bufs=2, space="PSUM") as psp,
        ):
            # ---- PE warm-up: a few matmuls on memset data so the HAM
            # clock-gate opens while the first DMAs land ----
            wup = const.tile([P, QC], BF16, tag="wup")
            nc.vector.memset(wup[:], 0)
            for i in range(4):
                wps = psp.tile([P, QC], F32, tag="pv", name=f"wps{i}")
                nc.tensor.matmul(wps[:], wup[:, 0:P], wup[:], start=True, stop=True)

            # ---- DMA prologue: wqk/xt(sc0) interleaved so the first
            # qk-projection can start as early as possible ----
            wqk_sb = const.tile([P, ND * 4 * P], BF16, tag="wqk")
            xt_sb = bigp.tile([P, ND * S], BF16, tag="xt")

            def xts(dt):
                return xt_sb[:, dt * S:(dt + 1) * S]

            def wqks(dt):
                return wqk_sb[:, dt * 4 * P:(dt + 1) * 4 * P]

            for dt in range(ND):
                nc.sync.dma_start(wqks(dt), wqk[dt * P:(dt + 1) * P, :])
                nc.sync.dma_start(
                    xts(dt)[:, 0:QC], xt[dt * P:(dt + 1) * P, 0:QC]
                )
            bqk_sb = const.tile([P, 4], F32, tag="bqk")
            nc.sync.dma_start(bqk_sb[:], bqk[:])
            wvb_sb = const.tile([1, VW], BF16, tag="wvb")
            nc.sync.dma_start(wvb_sb[:], wvb[:])
            wv_sb = const.tile([P, ND * VW], BF16, tag="wv")
            for dt in range(ND):
                nc.sync.dma_start(
                    wv_sb[:, dt * VW:(dt + 1) * VW], wv[dt * P:(dt + 1) * P, :]
                )
            tri_sb = const.tile([P, 2 * P], BF16, tag="tri")
            nc.sync.dma_start(tri_sb[:], tri[:])
            # xt chunk 1 early (vproj 4..7 is filler inside attention qt=0)
            for dt in range(ND):
                nc.sync.dma_start(
                    xts(dt)[:, QC:2 * QC], xt[dt * P:(dt + 1) * P, QC:2 * QC]
                )
            wp_sb = const.tile([P, 2 * D], BF16, tag="wp")
            for i in range(2):
                nc.sync.dma_start(
                    wp_sb[:, i * D:(i + 1) * D], wp[i * P:(i + 1) * P, :]
                )
            for sc in range(2, NQ):
                for dt in range(ND):
                    nc.sync.dma_start(
                        xts(dt)[:, sc * QC:(sc + 1) * QC],
                        xt[dt * P:(dt + 1) * P, sc * QC:(sc + 1) * QC],
                    )

            # v bias (+ ones columns) broadcast across partitions, once
            vbb_sb = const.tile([P, VW], BF16, tag="vbb")
            nc.gpsimd.partition_broadcast(vbb_sb[:], wvb_sb[:])

            # ---- persistent intermediates ----
            qt_sb = [bigp.tile([P, S], BF16, tag=f"qt{i}", name=f"qt{i}") for i in range(2)]
            kt_sb = [bigp.tile([P, S], BF16, tag=f"kt{i}", name=f"kt{i}") for i in range(2)]
            v_sb = bigp.tile([P, NK * VP], BF16, tag="v")
            ot_sb = [bigp.tile([P, S], BF16, tag=f"ot{i}", name=f"ot{i}") for i in range(2)]

            def qk_half(sc, et, half):
                """Half of one qk-projection tile (4 of 8 d-tiles)."""
                if half == 0:
                    qk_half.ps = psp.tile(
                        [P, QC], F32, tag="proj", name=f"qk{sc}{et}"
                    )
                ps = qk_half.ps
                for dt in range(4 * half, 4 * half + 4):
                    nc.tensor.matmul(
                        ps[:],
                        wqks(dt)[:, et * P:(et + 1) * P],
                        xts(dt)[:, sc * QC:(sc + 1) * QC],
                        start=(dt == 0),
                        stop=(dt == ND - 1),
                    )
                if half == 1:
                    dest = (qt_sb if et < 2 else kt_sb)[et % 2]
                    if sc <= 1:
                        # front of the kernel: exp stream is idle, DVE busy
                        nc.scalar.activation(
                            dest[:, sc * QC:(sc + 1) * QC], ps[:],
                            mybir.ActivationFunctionType.Identity,
                            bias=bqk_sb[:, et:et + 1],
                        )
                    else:
                        nc.vector.tensor_scalar_add(
                            dest[:, sc * QC:(sc + 1) * QC], ps[:],
                            bqk_sb[:, et:et + 1],
                        )

            def emit_v(st):
                ps = psp.tile([P, VW], F32, tag="proj", name=f"vp{st}")
                for dt in range(ND):
                    nc.tensor.matmul(
                        ps[:],
                        xts(dt)[:, st * P:(st + 1) * P],
                        wv_sb[:, dt * VW:(dt + 1) * VW],
                        start=(dt == 0),
                        stop=(dt == ND - 1),
                    )
                nc.vector.tensor_add(
                    v_sb[:, st * VP:st * VP + VW], ps[:], vbb_sb[:]
                )

            def mk_proj(qt, et, tag="proj", eng="v"):
                def f():
                    ps = psp.tile([P, QC], F32, tag=tag, name=f"yp{qt}{et}")
                    for i in range(2):
                        nc.tensor.matmul(
                            ps[:],
                            wp_sb[:, i * D + et * P:i * D + (et + 1) * P],
                            ot_sb[i][:, qt * QC:(qt + 1) * QC],
                            start=(i == 0),
                            stop=(i == 1),
                        )
                    ys = small.tile([P, QC], BF16, tag="ys", name=f"ys{qt}{et}",
                                    bufs=3)
                    if eng == "v":
                        nc.vector.tensor_copy(ys[:], ps[:])
                    else:
                        nc.scalar.copy(ys[:], ps[:])
                    nc.sync.dma_start(yt[et * P:(et + 1) * P, qt * QC:(qt + 1) * QC], ys[:])
                return f

            def emit_pair(qt, pair, fillers=()):
                """Attention for q-chunk qt, head-pair `pair` (2 heads)."""
                fillers = list(fillers)
                q0 = qt * QC
                nk = (qt + 1) * (QC // P)  # causal: k tiles 0..nk-1
                ht = pair
                pvs = [
                    psp.tile([P, QC], F32, tag="pv", name=f"pv{qt}{pair}{hh}")
                    for hh in range(2)
                ]
                exs = {}

                def emit_scores(kb):
                    j = kb - qt * (QC // P)
                    off = 0 if j < 0 else P * j
                    st2 = psp.tile(
                        [P, 2 * QC], F32, tag="sc", name=f"st{qt}{pair}{kb}"
                    )
                    for hh in range(2):
                        nc.tensor.matmul(
                            st2[:, hh * QC + off:(hh + 1) * QC],
                            kt_sb[ht][slice(64 * hh, 64 * hh + 64),
                                      kb * P:(kb + 1) * P],
                            qt_sb[ht][slice(64 * hh, 64 * hh + 64),
                                      q0 + off:q0 + QC],
                            start=True, stop=True,
                            tile_position=(64 * hh, 0),
                        )
                    ex = work.tile(
                        [P, 2 * QC], BF16, tag="ex", name=f"ex{qt}{pair}{kb}"
                    )
                    st3 = st2[:].rearrange("p (h q) -> p h q", h=2)[:, :, off:]
                    ex3 = ex[:].rearrange("p (h q) -> p h q", h=2)[:, :, off:]
                    nc.scalar.activation(
                        ex3, st3,
                        mybir.ActivationFunctionType.Exp,
                        scale=float(SCALE),
                    )
                    if j >= 0:
                        # only the 128-wide diagonal boundary needs masking
                        exb = ex[:].rearrange("p (h q) -> p h q", h=2)[:, :, off:off + P]
                        tri3 = tri_sb[:].rearrange("p (h q) -> p h q", h=2)
                        nc.vector.tensor_mul(exb, exb, tri3)
                    exs[kb] = ex

                def emit_pv(kb):
                    j = kb - qt * (QC // P)
                    off = 0 if j < 0 else P * j
                    ex = exs.pop(kb)
                    for hh in range(2):
                        h = 2 * pair + hh
                        nc.tensor.matmul(
                            pvs[hh][:, off:],
                            v_sb[:, kb * VP + h * (HD + 1):kb * VP + h * (HD + 1) + P],
                            ex[:, hh * QC + off:(hh + 1) * QC],
                            start=(kb == 0),
                            stop=(kb == nk - 1),
                        )

                # software pipeline: scores run 2 ahead of pv; one filler
                # after every other pv keeps the PE fed while exp paces
                emit_scores(0)
                if fillers:
                    fillers.pop(0)()
                emit_scores(1)
                for kb in range(nk):
                    if kb + 2 < nk:
                        emit_scores(kb + 2)
                    emit_pv(kb)
                    if fillers and kb % 2 == 0:
                        fillers.pop(0)()
                for f in fillers:
                    f()

                last = (qt, pair) == (NQ - 1, 1)
                dcps = []
                for hh in range(2):
                    # rows 0..63 are o^T, row 64 is the denominator
                    # (reciprocal_approx_fast misreads PSUM -> copy first)
                    dcp = small.tile(
                        [1, QC], F32, tag="dcp", name=f"dcp{qt}{pair}{hh}"
                    )
                    if last and hh == 0:
                        # final chain is fully exposed: run the two dcp
                        # copies on different engines in parallel
                        nc.scalar.copy(dcp[:], pvs[hh][HD:HD + 1, :])
                    else:
                        nc.vector.tensor_copy(dcp[:], pvs[hh][HD:HD + 1, :])
                    dcps.append(dcp)
                for hh in range(2):
                    rden = small.tile(
                        [1, QC], F32, tag="rden", name=f"rden{qt}{pair}{hh}"
                    )
                    nc.vector.reciprocal_approx_fast(rden[:], dcps[hh][:])
                    bden = small.tile(
                        [64, QC], F32, tag="bden", name=f"bden{qt}{pair}{hh}"
                    )
                    nc.gpsimd.partition_broadcast(bden[:], rden[:])
                    nc.vector.tensor_mul(
                        ot_sb[ht][slice(64 * hh, 64 * hh + 64), q0:q0 + QC],
                        pvs[hh][0:HD, :], bden[:],
                    )

            def qkproj(sc):
                for et in range(4):
                    qk_half(sc, et, 0)
                    qk_half(sc, et, 1)

            def mk_qk(sc, et, half):
                return lambda: qk_half(sc, et, half)

            # ---- causal streaming schedule ----
            # pair(x, 0) consumes et0/et2 (its own heads); its fillers
            # produce what the NEXT pair needs: pair(x,1) needs et1/et3,
            # pair(x+1,0) needs qkproj(x+1) et0/et2, plus v stripes and
            # outproj tiles of completed chunks.
            def mk_v(st):
                return lambda: emit_v(st)

            qkproj(0)
            for st in range(4):
                emit_v(st)
            emit_pair(0, 0, [mk_qk(1, et, h) for et in (0, 1) for h in (0, 1)]
                      + [mk_v(4)])
            emit_pair(0, 1, [mk_qk(1, et, h) for et in (2, 3) for h in (0, 1)]
                      + [mk_v(5)])
            emit_pair(1, 0,
                      [mk_v(6), mk_v(7)]
                      + [mk_qk(2, et, h) for et in (0, 1) for h in (0, 1)]
                      + [mk_proj(0, et) for et in range(2)])
            emit_pair(1, 1,
                      [mk_v(8), mk_v(9)]
                      + [mk_qk(2, et, h) for et in (2, 3) for h in (0, 1)]
                      + [mk_proj(0, et) for et in range(2, 4)])
            emit_pair(2, 0,
                      [mk_v(10), mk_v(11)]
                      + [mk_qk(3, et, h) for et in (0, 1) for h in (0, 1)]
                      + [mk_proj(0, et) for et in range(4, 6)]
                      + [mk_proj(1, et) for et in range(2)])
            emit_pair(2, 1,
                      [mk_v(12), mk_v(13)]
                      + [mk_qk(3, et, h) for et in (2, 3) for h in (0, 1)]
                      + [mk_proj(0, et) for et in range(6, 8)]
                      + [mk_proj(1, et) for et in range(2, 4)])
            emit_pair(3, 0,
                      [mk_v(14), mk_v(15)]
                      + [mk_proj(1, et) for et in range(4, 8)]
                      + [mk_proj(2, et) for et in range(2)])
            emit_pair(3, 1, [mk_proj(2, et) for et in range(2, 8)])
            # tail: the i=0 (first head-pair) halves of the first four
            # out-projection tiles run while the final normalize chain is
            # still draining; then finish with alternating psum tags and
            # drain engines so the last tiles stream back-to-back
            tail_ps = []
            for et in range(4):
                ps = psp.tile([P, QC], F32, tag=("proj", "sc")[et % 2],
                              name=f"yp3{et}")
                nc.tensor.matmul(
                    ps[:], wp_sb[:, et * P:(et + 1) * P],
                    ot_sb[0][:, 3 * QC:4 * QC], start=True, stop=False,
                )
                tail_ps.append(ps)
            for et in range(4):
                ps = tail_ps[et]
                nc.tensor.matmul(
                    ps[:], wp_sb[:, D + et * P:D + (et + 1) * P],
                    ot_sb[1][:, 3 * QC:4 * QC], start=False, stop=True,
                )
                ys = small.tile([P, QC], BF16, tag="ys", name=f"ys3{et}",
                                bufs=3)
                if et % 2 == 0:
                    nc.vector.tensor_copy(ys[:], ps[:])
                else:
                    nc.scalar.copy(ys[:], ps[:])
                nc.sync.dma_start(yt[et * P:(et + 1) * P, 3 * QC:4 * QC], ys[:])
            for et in range(4, 8):
                mk_proj(3, et, tag=("proj", "sc")[et % 2],
                        eng=("v", "s")[et % 2])()

    nc.compile()
    return nc


def _shard_inputs(x, w_qkv, b_qkv, w_proj):
    """Build the per-core input maps."""
    import ml_dtypes
    mdt = ml_dtypes.bfloat16

    in_maps = []
    kk = np.arange(P)[:, None]
    qq = np.arange(P)[None, :]
    tri_np = np.concatenate([(qq >= kk).astype(mdt)] * 2, axis=1)  # [128, 256]
    for c in range(NCORES):
        b, g = divmod(c, 4)
        e0 = g * HPC * HD  # 256*g
        xt_np = np.ascontiguousarray(x[b].T)
        q_rows = w_qkv[e0:e0 + HPC * HD]            # [256, 1024]
        k_rows = w_qkv[D + e0:D + e0 + HPC * HD]
        wqk_np = np.concatenate([q_rows.T, k_rows.T], 1)  # [1024, 512]
        wv_np = np.zeros((D, VW), np.float32)
        wvb_np = np.zeros((1, VW), np.float32)
        for h in range(HPC):
            rows = 2 * D + e0 + h * HD
            wv_np[:, h * (HD + 1):h * (HD + 1) + HD] = w_qkv[rows:rows + HD].T
            wvb_np[0, h * (HD + 1):h * (HD + 1) + HD] = b_qkv[rows:rows + HD]
            wvb_np[0, h * (HD + 1) + HD] = 1.0
        bqk_np = np.stack(
            [b_qkv[e0:e0 + P], b_qkv[e0 + P:e0 + 2 * P],
             b_qkv[D + e0:D + e0 + P], b_qkv[D + e0 + P:D + e0 + 2 * P]], 1
        ).astype(np.float32)
        wp_np = np.ascontiguousarray(w_proj[:, e0:e0 + HPC * HD].T)  # [256, 1024]
        in_maps.append({
            "xt": np.ascontiguousarray(xt_np.astype(mdt)),
            "wqk": np.ascontiguousarray(wqk_np.astype(mdt)),
            "bqk": np.ascontiguousarray(bqk_np),
            "wv": wv_np.astype(mdt),
            "wvb": wvb_np.astype(mdt),
            "tri": tri_np,
            "wp": wp_np.astype(mdt),
        })
    return in_maps


def _run(inputs, trace=False, trace_kwargs=None):
    global _PROGRAM
    if _PROGRAM is None:
        _PROGRAM = _build_program()
    nc = _PROGRAM
    x = np.asarray(inputs["x"], np.float32)
    w_qkv = np.asarray(inputs["w_qkv"], np.float32)
    b_qkv = np.asarray(inputs["b_qkv"], np.float32)
    w_proj = np.asarray(inputs["w_proj"], np.float32)
    b_proj = np.asarray(inputs["b_proj"], np.float32)
    in_maps = _shard_inputs(x, w_qkv, b_qkv, w_proj)
    res = run_bass_kernel_spmd(
        nc, in_maps, core_ids=list(range(NCORES)),
        trace=trace, **(trace_kwargs or {}),
    )
    y = np.zeros((B, S, D), np.float32)
    for c in range(NCORES):
        y[c // 4] += res.results[c]["yt"].T.astype(np.float32)
    y += b_proj
    return y, res


def kernel(**inputs):
    y, _ = _run(inputs)
    return y
